# revision 3
# baseline (speedup 1.0000x reference)
"""MatchLSTM Trainium2 kernel v7: data-parallel over batch (8 cores, 1 elem each).

Column-layout recurrences: hidden state lives as SBUF column chunks
(A=[0:128], B=[128:150]); every per-step matmul has a free-size-1 output
(h-projections as column projections, x-injections via identity lhsT),
gate nonlinearities run as per-partition activation ops with fused
scale (reset gate) and bias (input projection), and the GRU combine is two
scalar_tensor_tensor ops writing the next h column in place.  The match
attention uses tanh(whqT + bias=wr_col) ([*,64] activations), an
attn = GT^T w column matmul, and x2 = HqW2^T attn column matmuls.
ctx(t) and match(t-2) are software-pipelined in one slot loop.
"""
import math
from contextlib import ExitStack

import numpy as np
import ml_dtypes

import concourse.bacc as bacc
import concourse.bass as bass
import concourse.mybir as mybir
import concourse.tile as tile
from concourse.bass_utils import run_bass_kernel_spmd

F32 = mybir.dt.float32
BF16 = mybir.dt.bfloat16
I32 = mybir.dt.int32
AF = mybir.ActivationFunctionType
OP = mybir.AluOpType
BF = ml_dtypes.bfloat16

H = 150
D = 300
J = 64
V = 100000
HA, HB = 128, 22
# gate chunks within the 450-wide (r|z|n) projection, gate-aligned
GCH = [(0, 128), (128, 22), (150, 128), (278, 22), (300, 128), (428, 22)]
# psum col order: 0=rA 1=rB 2=zA 3=zB 4=nA 5=nB


def build(T=400):
    NT = math.ceil(T / 128)
    tsz = [min(128, T - 128 * g) for g in range(NT)]

    nc = bacc.Bacc("TRN2", target_bir_lowering=False, debug=False, num_devices=8)

    dram = {}

    def din(name, shape, dt):
        dram[name] = nc.dram_tensor(name, list(shape), dt, kind="ExternalInput")
        return dram[name]

    din("E", [V, D], F32)
    din("ctx_idx", [128, NT], I32)
    din("q_idx", [J, 1], I32)
    din("Ifp", [128, 128], F32)
    din("Ibf", [128, 128], BF16)
    wspec = []
    for g in ("c", "q"):
        wspec += [(f"WihT_{g}_0", (128, 450)), (f"WihT_{g}_1", (128, 450)),
                  (f"WihT_{g}_2", (45, 450))]
    for g in ("c", "q", "m"):
        wspec += [(f"WhhT_{g}_A", (128, 450)), (f"WhhT_{g}_B", (23, 450))]
    wspec += [("WcT_A", (128, 450)), ("WcT_B", (22, 450)),
              ("W2T_A", (128, 450)), ("W2T_B", (22, 450)),
              ("WrA", (128, H)), ("WrB", (22, H)),
              ("WpA", (128, H)), ("WpB", (22, H)),
              ("WqA", (128, H)), ("WqB", (22, H)),
              ("mbih", (1, 450)), ("ones_row", (1, 512)),
              ("wcolA", (128, 1)), ("wcolB", (22, 1))]
    for n, s in wspec:
        din(n, s, BF16)
    hrA_d = nc.dram_tensor("hrA", [128, T + 1], BF16, kind="ExternalOutput")
    hrB_d = nc.dram_tensor("hrB", [22, T + 1], BF16, kind="ExternalOutput")

    with tile.TileContext(nc) as tc, ExitStack() as st:
        sb = st.enter_context(tc.tile_pool(name="sb", bufs=1))

        def sbt(name, shape, dt):
            return sb.tile(list(shape), dt, tag=name, name=name)

        W = {n: sbt(n, s, BF16) for n, s in wspec}
        Ifp = sbt("Ifp", (128, 128), F32)
        Ibf = sbt("Ibf", (128, 128), BF16)
        cidx = sbt("cidx", (128, NT), I32)
        qidx = sbt("qidx", (J, 1), I32)
        ec = [sbt(f"ec{g}", (128, D), F32) for g in range(NT)]
        eq = sbt("eq", (J, D), F32)
        ecT = [sbt("ecT0", (128, T), BF16), sbt("ecT1", (128, T), BF16),
               sbt("ecT2", (45, T), BF16)]
        eqT = [sbt("eqT0", (128, J), BF16), sbt("eqT1", (128, J), BF16),
               sbt("eqT2", (45, J), BF16)]
        XPc = [sbt(f"XPc{i}", (gsz, T), BF16) for i, (go, gsz) in enumerate(GCH)]
        XPq = [sbt(f"XPq{i}", (gsz, J), BF16) for i, (go, gsz) in enumerate(GCH)]
        HcA = sbt("HcA", (128, T + 1), BF16)
        HcB = sbt("HcB", (23, T + 1), BF16)
        HqA = sbt("HqA", (128, J + 1), BF16)
        HqB = sbt("HqB", (23, J + 1), BF16)
        HmA = sbt("HmA", (128, T + 1), BF16)
        HmB = sbt("HmB", (23, T + 1), BF16)
        whqT_A = sbt("whqT_A", (128, J), BF16)
        whqT_B = sbt("whqT_B", (22, J), BF16)
        HqW2 = sbt("HqW2", (J + 1, 450), BF16)
        attn_sb = sbt("attn_sb", (J + 1, 1), BF16)
        GT_A = sbt("GT_A", (128, J), BF16)
        GT_B = sbt("GT_B", (22, J), BF16)
        wrA = sbt("wrA", (128, 1), F32)
        wrB = sbt("wrB", (22, 1), F32)
        xnA = sbt("xnA", (128, 1), F32)
        xnB = sbt("xnB", (22, 1), F32)
        bias_g = {i: sbt(f"biasg{i}", (gsz, 1), F32)
                  for i, (go, gsz) in enumerate(GCH[:4])}
        # per-GRU gate scratch (column chunks)
        gs = {}
        for g in ("q", "c", "m"):
            gs[g] = dict(
                rA=sbt(f"rA_{g}", (128, 1), F32), rB=sbt(f"rB_{g}", (22, 1), F32),
                zA=sbt(f"zA_{g}", (128, 1), F32), zB=sbt(f"zB_{g}", (22, 1), F32),
                nA=sbt(f"nA_{g}", (128, 1), F32), nB=sbt(f"nB_{g}", (22, 1), F32),
                tA=sbt(f"tA_{g}", (128, 1), F32), tB=sbt(f"tB_{g}", (22, 1), F32),
                **{f"x2_{i}": sbt(f"x2_{g}{i}", (gsz2, 1), F32)
                   for i, (go2, gsz2) in enumerate(GCH)},
            )

        # ---- load constants / weights / indices ----
        for n, _ in wspec:
            nc.sync.dma_start(W[n][:], dram[n].ap())
        nc.sync.dma_start(Ifp[:], dram["Ifp"].ap())
        nc.sync.dma_start(Ibf[:], dram["Ibf"].ap())
        nc.sync.dma_start(cidx[:], dram["ctx_idx"].ap())
        nc.sync.dma_start(qidx[:], dram["q_idx"].ap())
        nc.sync.dma_start(HqW2[J:J + 1, :], dram["mbih"].ap())

        # ---- init state ----
        ones_ap = dram["ones_row"].ap()
        for hA, hB, ncol in ((HcA, HcB, T + 1), (HqA, HqB, J + 1),
                             (HmA, HmB, T + 1)):
            nc.vector.memset(hA[:, 0:1], 0.0)
            nc.vector.memset(hB[0:22, 0:1], 0.0)
            nc.sync.dma_start(hB[22:23, 0:ncol], ones_ap[0:1, 0:ncol])
        nc.sync.dma_start(attn_sb[J:J + 1, 0:1], ones_ap[0:1, 0:1])
        nc.sync.dma_start(ecT[2][44:45, 0:T], ones_ap[0:1, 0:T])
        nc.sync.dma_start(eqT[2][44:45, 0:J], ones_ap[0:1, 0:J])

        # ---- gathers ----
        for g in range(NT):
            nc.gpsimd.indirect_dma_start(
                out=ec[g][:], out_offset=None, in_=dram["E"].ap(),
                in_offset=bass.IndirectOffsetOnAxis(ap=cidx[:, g:g + 1], axis=0))
        nc.gpsimd.indirect_dma_start(
            out=eq[:], out_offset=None, in_=dram["E"].ap(),
            in_offset=bass.IndirectOffsetOnAxis(ap=qidx[:, 0:1], axis=0))

        dch = [(0, 128), (128, 128), (256, 44)]

        # ---- preamble: transposes + input projections ----
        with tc.tile_pool(name="pre_ps", bufs=2, space="PSUM") as pps, \
             tc.tile_pool(name="xp_ps", bufs=2, space="PSUM") as xps:
            for g in range(NT):
                toff = 128 * g
                for k, (doff, dsz) in enumerate(dch):
                    tp = pps.tile([128, 128], F32, tag="tp", name="tp")
                    nc.tensor.transpose(tp[0:dsz, 0:tsz[g]],
                                        ec[g][0:tsz[g], doff:doff + dsz],
                                        Ifp[0:tsz[g], 0:tsz[g]])
                    nc.scalar.copy(ecT[k][0:dsz, toff:toff + tsz[g]],
                                   tp[0:dsz, 0:tsz[g]])
            for k, (doff, dsz) in enumerate(dch):
                tp = pps.tile([128, 128], F32, tag="tp", name="tp")
                nc.tensor.transpose(tp[0:dsz, 0:J], eq[0:J, doff:doff + dsz],
                                    Ifp[0:J, 0:J])
                nc.scalar.copy(eqT[k][0:dsz, 0:J], tp[0:dsz, 0:J])
            # XPc/XPq gate-chunk tiles (transposed input projections)
            for i, (go, gsz) in enumerate(GCH):
                px = xps.tile([128, T], F32, tag="px", name="px")
                for k in range(3):
                    ksz = [128, 128, 45][k]
                    nc.tensor.matmul(px[0:gsz, 0:T],
                                     W[f"WihT_c_{k}"][0:ksz, go:go + gsz],
                                     ecT[k][0:ksz, 0:T],
                                     start=(k == 0), stop=(k == 2))
                nc.vector.tensor_copy(XPc[i][:], px[0:gsz, 0:T])
            for i, (go, gsz) in enumerate(GCH):
                px = xps.tile([128, J], F32, tag="pxq", name="pxq")
                for k in range(3):
                    ksz = [128, 128, 45][k]
                    nc.tensor.matmul(px[0:gsz, 0:J],
                                     W[f"WihT_q_{k}"][0:ksz, go:go + gsz],
                                     eqT[k][0:ksz, 0:J],
                                     start=(k == 0), stop=(k == 2))
                nc.scalar.copy(XPq[i][:], px[0:gsz, 0:J])

        psp = st.enter_context(tc.tile_pool(name="cell_ps", bufs=3, space="PSUM"))

        def gru_cell(g, t, XT, HA_, HB_, WhA, WhB):
            """One column-layout GRU cell: reads h at col t, writes col t+1."""
            s = gs[g]
            ps = psp.tile([128, 24], F32, tag="ps_cell", name=f"ps_{g}")
            hA_ap = HA_[:, t:t + 1]
            hB_ap = HB_[:, t:t + 1]
            for col, (go, gsz) in enumerate(GCH):
                nc.tensor.matmul(ps[0:gsz, col:col + 1], WhA[:, go:go + gsz],
                                 hA_ap, start=True, stop=False)
                nc.tensor.matmul(ps[0:gsz, col:col + 1], WhB[:, go:go + gsz],
                                 hB_ap, start=False, stop=(col >= 4))
            for col in range(4):
                gsz = GCH[col][1]
                nc.tensor.matmul(ps[0:gsz, col:col + 1], Ibf[0:gsz, 0:gsz],
                                 XT[col][:, t:t + 1], start=False, stop=True)
            nc.scalar.activation(s["rA"][:], ps[0:128, 0:1], AF.Sigmoid)
            nc.scalar.activation(s["rB"][:], ps[0:22, 1:2], AF.Sigmoid)
            nc.scalar.activation(s["nA"][:], ps[0:128, 4:5], AF.Tanh,
                                 scale=s["rA"][:], bias=XT[4][:, t:t + 1])
            nc.scalar.activation(s["nB"][:], ps[0:22, 5:6], AF.Tanh,
                                 scale=s["rB"][:], bias=XT[5][:, t:t + 1])
            nc.scalar.activation(s["zA"][:], ps[0:128, 2:3], AF.Sigmoid)
            nc.scalar.activation(s["zB"][:], ps[0:22, 3:4], AF.Sigmoid)
            # h2 = z*h + (1-z)*n  via  t=(n*z)-n ; h2=(h*z)-t
            nc.vector.scalar_tensor_tensor(out=s["tA"][:], in0=s["nA"][:],
                                           scalar=s["zA"][:], in1=s["nA"][:],
                                           op0=OP.mult, op1=OP.subtract)
            nc.vector.scalar_tensor_tensor(out=HA_[:, t + 1:t + 2], in0=hA_ap,
                                           scalar=s["zA"][:], in1=s["tA"][:],
                                           op0=OP.mult, op1=OP.subtract)
            nc.vector.scalar_tensor_tensor(out=s["tB"][:], in0=s["nB"][:],
                                           scalar=s["zB"][:], in1=s["nB"][:],
                                           op0=OP.mult, op1=OP.subtract)
            nc.vector.scalar_tensor_tensor(out=HB_[0:22, t + 1:t + 2],
                                           in0=hB_ap[0:22, :],
                                           scalar=s["zB"][:], in1=s["tB"][:],
                                           op0=OP.mult, op1=OP.subtract)

        # ---- q-GRU ----
        for j in range(J):
            gru_cell("q", j, XPq, HqA, HqB, W["WhhT_q_A"], W["WhhT_q_B"])

        # ---- whqT, HqW2 ----
        with tc.tile_pool(name="wq_ps", bufs=1, space="PSUM") as wqp:
            pAB = wqp.tile([128, 2 * J], F32, tag="pwq", name="pwq")
            pA = pAB[:, 0:J]
            pB = pAB[0:22, J:2 * J]
            pW = wqp.tile([J, 450], F32, tag="pW2", name="pW2")
            nc.tensor.matmul(pA, W["WqA"][:, 0:128], HqA[:, 1:J + 1],
                             start=True, stop=False)
            nc.tensor.matmul(pA, W["WqB"][:, 0:128],
                             HqB[0:22, 1:J + 1], start=False, stop=True)
            nc.scalar.copy(whqT_A[:], pA)
            nc.tensor.matmul(pB, W["WqA"][:, 128:150], HqA[:, 1:J + 1],
                             start=True, stop=False)
            nc.tensor.matmul(pB, W["WqB"][:, 128:150],
                             HqB[0:22, 1:J + 1], start=False, stop=True)
            nc.scalar.copy(whqT_B[:], pB)
            nc.tensor.matmul(pW[0:J, :], HqA[:, 1:J + 1], W["W2T_A"][:],
                             start=True, stop=False)
            nc.tensor.matmul(pW[0:J, :], HqB[0:22, 1:J + 1], W["W2T_B"][:],
                             start=False, stop=True)
            nc.scalar.copy(HqW2[0:J, :], pW[0:J, :])

        def match_cell(m):
            # pm col map: 0..5 hproj (rA rB zA zB nA nB), 6..11 x2 (same
            # order), 12..17 zx (same order), 18 attn, 19 wrA, 20 wrB.
            # Each accumulation group holds matmuls of one dependency class
            # only; DVE adds reunify zx+x2 into SBUF activation biases.
            s = gs["m"]
            pm = psp.tile([128, 24], F32, tag="ps_cell", name="pm")
            pwr = pm[:, 19:21]
            pat = pm[0:J, 18:19]
            hmA = HmA[:, m:m + 1]
            hmB = HmB[:, m:m + 1]           # [23,1] incl bias row
            hcA = HcA[:, m + 1:m + 2]
            hcB = HcB[0:22, m + 1:m + 2]    # no bias row
            # wr = Wr@hm + Wp@hc   (column chunks A,B -> pwr cols 0,1)
            for c, (po, psz2) in enumerate(((0, 128), (128, 22))):
                nc.tensor.matmul(pwr[0:psz2, c:c + 1], W["WrA"][:, po:po + psz2],
                                 hmA, start=True, stop=False)
                nc.tensor.matmul(pwr[0:psz2, c:c + 1], W["WrB"][:, po:po + psz2],
                                 hmB[0:22, :], start=False, stop=False)
                nc.tensor.matmul(pwr[0:psz2, c:c + 1], W["WpA"][:, po:po + psz2],
                                 hcA, start=False, stop=False)
                nc.tensor.matmul(pwr[0:psz2, c:c + 1], W["WpB"][:, po:po + psz2],
                                 hcB, start=False, stop=True)
            # gate psums: cols 0..5 = gates, 6..7 = xn (x-part of n gate)
            # r/z cols 0..3: hproj + zx share the column (two dependency
            # classes, the same shape the wr group uses); n cols 4,5 hold
            # hproj only (tanh scale multiplies the whole column); zxn gets
            # its own cols 12,13.
            for col, (go, gsz) in enumerate(GCH):
                nc.tensor.matmul(pm[0:gsz, col:col + 1],
                                 W["WhhT_m_A"][:, go:go + gsz], hmA,
                                 start=True, stop=False)
                nc.tensor.matmul(pm[0:gsz, col:col + 1],
                                 W["WhhT_m_B"][:, go:go + gsz], hmB,
                                 start=False, stop=(col >= 4))
            for col in range(4):
                go, gsz = GCH[col]
                nc.tensor.matmul(pm[0:gsz, col:col + 1],
                                 W["WcT_A"][:, go:go + gsz], hcA,
                                 start=False, stop=False)
                nc.tensor.matmul(pm[0:gsz, col:col + 1],
                                 W["WcT_B"][:, go:go + gsz], hcB,
                                 start=False, stop=True)
            for xc in range(2):
                go, gsz = GCH[4 + xc]
                nc.tensor.matmul(pm[0:gsz, 12 + xc:13 + xc],
                                 W["WcT_A"][:, go:go + gsz], hcA,
                                 start=True, stop=False)
                nc.tensor.matmul(pm[0:gsz, 12 + xc:13 + xc],
                                 W["WcT_B"][:, go:go + gsz], hcB,
                                 start=False, stop=True)
            # wr -> sbuf (bias for tanhG)
            nc.vector.tensor_copy(wrA[:], pwr[0:128, 0:1])
            nc.vector.tensor_copy(wrB[:], pwr[0:22, 1:2])
            # GT = tanh(whqT + wr)
            nc.scalar.activation(GT_A[:], whqT_A[:], AF.Tanh, bias=wrA[:])
            nc.scalar.activation(GT_B[:], whqT_B[:], AF.Tanh, bias=wrB[:])
            # attn = GT^T w  (column)
            nc.tensor.matmul(pat, GT_A[:], W["wcolA"][:],
                             start=True, stop=False)
            nc.tensor.matmul(pat, GT_B[:], W["wcolB"][:],
                             start=False, stop=True)
            nc.vector.tensor_copy(attn_sb[0:J, 0:1], pat)
            # x2 = HqW2^T @ attn (+ m_bih via attn[J]=1)
            for col, (go, gsz) in enumerate(GCH):
                nc.tensor.matmul(pm[0:gsz, 6 + col:7 + col],
                                 HqW2[:, go:go + gsz], attn_sb[:],
                                 start=True, stop=True)
            # x2 -> SBUF copies; r/z sig bias = x2 directly; n bias = zxn + x2n
            x2sb = {}
            for col, (go, gsz) in enumerate(GCH):
                x2sb[col] = gs["m"]["x2_%d" % col]
                nc.vector.tensor_copy(x2sb[col][:], pm[0:gsz, 6 + col:7 + col])
            nc.vector.tensor_tensor(out=xnA[:], in0=pm[0:128, 12:13],
                                    in1=x2sb[4][:], op=OP.add)
            nc.vector.tensor_tensor(out=xnB[:], in0=pm[0:22, 13:14],
                                    in1=x2sb[5][:], op=OP.add)
            # gates: sigma(hproj + bias), tanh(hn*r + xn)
            nc.scalar.activation(s["rA"][:], pm[0:128, 0:1], AF.Sigmoid,
                                 bias=x2sb[0][:])
            nc.scalar.activation(s["rB"][:], pm[0:22, 1:2], AF.Sigmoid,
                                 bias=x2sb[1][:])
            nc.scalar.activation(s["nA"][:], pm[0:128, 4:5], AF.Tanh,
                                 scale=s["rA"][:], bias=xnA[:])
            nc.scalar.activation(s["nB"][:], pm[0:22, 5:6], AF.Tanh,
                                 scale=s["rB"][:], bias=xnB[:])
            nc.scalar.activation(s["zA"][:], pm[0:128, 2:3], AF.Sigmoid,
                                 bias=x2sb[2][:])
            nc.scalar.activation(s["zB"][:], pm[0:22, 3:4], AF.Sigmoid,
                                 bias=x2sb[3][:])
            nc.vector.scalar_tensor_tensor(out=s["tA"][:], in0=s["nA"][:],
                                           scalar=s["zA"][:], in1=s["nA"][:],
                                           op0=OP.mult, op1=OP.subtract)
            nc.vector.scalar_tensor_tensor(out=HmA[:, m + 1:m + 2], in0=hmA,
                                           scalar=s["zA"][:], in1=s["tA"][:],
                                           op0=OP.mult, op1=OP.subtract)
            nc.vector.scalar_tensor_tensor(out=s["tB"][:], in0=s["nB"][:],
                                           scalar=s["zB"][:], in1=s["nB"][:],
                                           op0=OP.mult, op1=OP.subtract)
            nc.vector.scalar_tensor_tensor(out=HmB[0:22, m + 1:m + 2],
                                           in0=hmB[0:22, :],
                                           scalar=s["zB"][:], in1=s["tB"][:],
                                           op0=OP.mult, op1=OP.subtract)

        # ---- main slot loop: ctx(t) || match(t-2) ----
        for t in range(T + 2):
            if t < T:
                gru_cell("c", t, XPc, HcA, HcB, W["WhhT_c_A"], W["WhhT_c_B"])
            if t >= 2:
                match_cell(t - 2)

        # ---- output ----
        nc.sync.dma_start(hrA_d.ap(), HmA[:])
        nc.sync.dma_start(hrB_d.ap(), HmB[0:22, :])

    nc.compile()
    return nc


def _bf(x):
    return np.ascontiguousarray(np.asarray(x, np.float32)).astype(BF)


def prep_shared(E, Wq, Wp, Wr, w, ctx_Wih, ctx_Whh, ctx_bih, ctx_bhh,
                q_Wih, q_Whh, q_bih, q_bhh, m_Wih, m_Whh, m_bih, m_bhh):
    f = {}
    f["Ifp"] = np.eye(128, dtype=np.float32)
    f["Ibf"] = _bf(np.eye(128))

    def wih_chunks(pfx, Wih, bih):
        WT = np.asarray(Wih, np.float32).T  # [300, 450]
        f[f"WihT_{pfx}_0"] = _bf(WT[0:128])
        f[f"WihT_{pfx}_1"] = _bf(WT[128:256])
        f[f"WihT_{pfx}_2"] = _bf(np.vstack([WT[256:300],
                                            np.asarray(bih, np.float32)[None, :]]))

    def whh_chunks(pfx, Whh, bhh):
        WT = np.asarray(Whh, np.float32).T  # [150, 450]
        f[f"WhhT_{pfx}_A"] = _bf(WT[0:128])
        f[f"WhhT_{pfx}_B"] = _bf(np.vstack([WT[128:150],
                                            np.asarray(bhh, np.float32)[None, :]]))

    wih_chunks("q", q_Wih, q_bih)
    wih_chunks("c", ctx_Wih, ctx_bih)
    whh_chunks("q", q_Whh, q_bhh)
    whh_chunks("c", ctx_Whh, ctx_bhh)
    whh_chunks("m", m_Whh, m_bhh)
    m_Wih = np.asarray(m_Wih, np.float32)
    WcT = m_Wih[:, :H].T  # [150, 450]
    f["WcT_A"] = _bf(WcT[0:128])
    f["WcT_B"] = _bf(WcT[128:150])
    W2T = m_Wih[:, H:].T  # [150, 450]
    f["W2T_A"] = _bf(W2T[0:128])
    f["W2T_B"] = _bf(W2T[128:150])
    for nm, M in (("Wr", Wr), ("Wp", Wp), ("Wq", Wq)):
        M = np.asarray(M, np.float32)  # [150, 150]
        f[f"{nm}A"] = _bf(M[0:128])
        f[f"{nm}B"] = _bf(M[128:150])
    f["mbih"] = _bf(np.asarray(m_bih, np.float32)[None, :])
    f["ones_row"] = _bf(np.ones((1, 512)))
    wv = np.asarray(w, np.float32)
    f["wcolA"] = _bf(wv[0:128, None])
    f["wcolB"] = _bf(wv[128:150, None])
    return f


_NC_CACHE = {}


def kernel(context, query, E, Wq, Wp, Wr, w, ctx_Wih, ctx_Whh, ctx_bih,
           ctx_bhh, q_Wih, q_Whh, q_bih, q_bhh, m_Wih, m_Whh, m_bih, m_bhh,
           _T=None):
    context = np.asarray(context)
    query = np.asarray(query)
    B, T = context.shape
    if _T is not None:
        T = _T
        context = context[:, :T]
    NT = math.ceil(T / 128)
    if T not in _NC_CACHE:
        _NC_CACHE[T] = build(T)
    nc = _NC_CACHE[T]

    shared = prep_shared(E, Wq, Wp, Wr, w, ctx_Wih, ctx_Whh, ctx_bih, ctx_bhh,
                         q_Wih, q_Whh, q_bih, q_bhh, m_Wih, m_Whh, m_bih, m_bhh)
    E_np = np.ascontiguousarray(np.asarray(E, np.float32))
    in_maps = []
    for b in range(B):
        m = dict(shared)
        m["E"] = E_np
        ci = np.zeros((128, NT), np.int32)
        flat = np.asarray(context[b], np.int64).astype(np.int32)
        for g in range(NT):
            n = min(128, T - 128 * g)
            ci[0:n, g] = flat[128 * g:128 * g + n]
        m["ctx_idx"] = ci
        m["q_idx"] = np.asarray(query[b], np.int64).astype(np.int32)[:, None]
        in_maps.append(m)

    res = run_bass_kernel_spmd(nc, in_maps, core_ids=list(range(B)))
    out = []
    for r in res.results:
        full = np.concatenate([np.asarray(r["hrA"], np.float32),
                               np.asarray(r["hrB"], np.float32)], axis=0)
        out.append(full.T)  # [T+1, 150]
    return np.stack(out, axis=0).astype(np.float32)
